# revision 67
# baseline (speedup 1.0000x reference)
"""Trainium2 Bass kernel for the Mamba-style block nn_Block_29721173688983.

Strategy: tensor-parallel over d_inner (2048 channels / 8 cores = 256 each).
Per core, one fused per-512-column pipeline: RMSNorm (redundant; sumsq via
bf16 DVE squares + ones-matmul, sqrt on Act, reciprocal on DVE, partition
broadcast via PE ones-matmul), bf16 w_in column-slice matmul, depthwise conv
as diagonal bf16 matmuls + silu, partial x_proj contraction -> two chunked
bf16 AllReduces (cc0 after half the sequence, pipelined with the rest).
Phase C per 1024-chunk: delta via softplus = z + ln(1+exp(-z)) (Exp/Ln share
one Act table set with the scan's exp), selective scan via DVE
tensor_tensor_scan (f32 da, bf16 dbx/ys, state chained across chunks),
z = ys*C split DVE/Pool, y = sum_n via bf16 identity-matmul PSUM
accumulation consumed per-chunk, fin + out_proj bf16 matmul streamed per
chunk.  Host sums the 8 bf16 partial outputs in f32 and adds the residual.

kernel(**inputs) takes the FULL unsharded inputs from setup_inputs() and
returns the FULL (1, 2048, 1024) output.
"""

import sys

sys.path.insert(0, "/opt/trn_rl_repo")

from contextlib import ExitStack

import numpy as np

import concourse.bacc as bacc
import concourse.bass as bass
import concourse.tile as tile
from concourse import mybir
from concourse.bass_utils import run_bass_kernel_spmd

F32 = mybir.dt.float32
F32R = mybir.dt.float32r
BF16 = mybir.dt.bfloat16
AF = mybir.ActivationFunctionType
OP = mybir.AluOpType

CORES = 8
D = 1024
DI = 2048
CL = DI // CORES          # 256 channels per core
NB = CL // 128            # 2 channel blocks
NST = 16                  # d_state
DTR = 64                  # dt_rank
KCONV = 4
EPS = 1e-5
NPROJ = DTR + 2 * NST     # 96


class _StopBuild(Exception):
    pass


def build(L=2048, collective=True, stop_after="D", reps=1, pool_mod=6):
    """Emit the SPMD single-core program (same program on all 8 cores).

    pool_mod: z-mult (n,cb) goes to Pool unless idx % pool_mod == pool_mod-1.
    """
    LTS = 512                  # psum free-dim tile
    NLT = L // LTS             # 4
    KB = D // 128              # 8 k-blocks for the w_in matmul
    LC = min(1024, L)          # collective + scan chunk
    NCH = L // LC              # 2
    LPC = LC // LTS            # psum tiles per chunk (2)
    LTPC = LC // LTS           # lt chunks per collective chunk (2)

    nc = bacc.Bacc("TRN2", target_bir_lowering=False, debug=False,
                   num_devices=CORES if collective else 1)

    def din(name, shape, dt=F32):
        return nc.dram_tensor(name, shape, dt, kind="ExternalInput").ap()

    xT_d = din("xT", [D, L], BF16)
    w_in_d = din("w_in_pack", [128, KB * 512], BF16)        # [p, (kb, m*128)]
    cbias_d = din("cbias_pack", [128, NB])
    A_d = din("A_pack", [128, NB * NST])
    D_d = din("D_pack", [128, NB])
    wxp_d = din("wxp_pack", [128, NB * NPROJ], BF16)
    wdt_d = din("wdt_loc", [DTR, CL], BF16)
    bdt_d = din("bdt_pack", [128, NB])
    bdtn_d = din("bdtn_pack", [128, NB])
    wout_d = din("wout_pack", [128, NB * D], BF16)
    ident_d = din("ident", [128, 128], BF16)
    ones_d = din("ones_in", [128, 1], BF16)
    cwdiag_d = din("cwdiag_pack", [128, NB * KCONV * 128], BF16)
    ddiag_d = din("ddiag_pack", [128, NB * 128], BF16)

    pout_d = nc.dram_tensor("part_out", [D, L], BF16, kind="ExternalOutput").ap()

    with tile.TileContext(nc) as tc:
      with ExitStack() as ctx:
        try:
            # ---- persistent pools ----
            cpool = ctx.enter_context(tc.tile_pool(name="consts", bufs=1))
            drpool = ctx.enter_context(
                tc.tile_pool(name="dram", bufs=1, space="DRAM"))

            cbias_t = cpool.tile([128, NB], F32)
            A_t = cpool.tile([128, NB * NST], F32)
            Aneg_t = cpool.tile([128, NB * NST], F32)
            D_t = cpool.tile([128, NB], F32)
            wxp_t = cpool.tile([128, NB * NPROJ], BF16)
            wdt_t = cpool.tile([DTR, CL], BF16)
            bdt_t = cpool.tile([128, NB], F32)
            bdtn_t = cpool.tile([128, NB], F32)
            wout_t = cpool.tile([128, NB * D], BF16)
            ident_t = cpool.tile([128, 128], BF16)
            ones_t = cpool.tile([128, 1], BF16)
            onesr_t = cpool.tile([1, 128], BF16)
            eps_t = cpool.tile([1, 1], F32)
            cwdiag_t = cpool.tile([128, NB * KCONV * 128], BF16)
            ddiag_t = cpool.tile([128, NB * 128], BF16)
            w_in_t = cpool.tile([128, KB * 512], BF16)

            nc.sync.dma_start(cbias_t[:], cbias_d[:])
            nc.sync.dma_start(A_t[:], A_d[:])
            nc.sync.dma_start(D_t[:], D_d[:])
            nc.sync.dma_start(wxp_t[:], wxp_d[:])
            nc.sync.dma_start(wdt_t[:], wdt_d[:])
            nc.sync.dma_start(bdt_t[:], bdt_d[:])
            nc.sync.dma_start(bdtn_t[:], bdtn_d[:])
            nc.sync.dma_start(wout_t[:], wout_d[:])
            nc.sync.dma_start(ident_t[:], ident_d[:])
            nc.sync.dma_start(ones_t[:], ones_d[:])
            nc.sync.dma_start(cwdiag_t[:], cwdiag_d[:])
            nc.sync.dma_start(ddiag_t[:], ddiag_d[:])
            nc.sync.dma_start(w_in_t[:, 0:KB * 256], w_in_d[:, 0:KB * 256])
            nc.sync.dma_start(w_in_t[:, KB * 256:], w_in_d[:, KB * 256:])

            nc.vector.memset(eps_t[:], EPS)
            nc.vector.memset(onesr_t[:], 1.0)

            # A = -exp(A_log)
            nc.scalar.activation(Aneg_t[:], A_t[:], AF.Exp)
            nc.scalar.mul(Aneg_t[:], Aneg_t[:], -1.0)

            for rep in range(reps):
              with ExitStack() as rctx:
                # per-rep persistent activations (live through phase C)
                ppool = rctx.enter_context(
                    tc.tile_pool(name=f"persist{rep}", bufs=1))
                # symmetric collective chunks: cc0 = lt0..1, cc1 = lt2..3
                CCW = [LC, LC]
                ar_in = [drpool.tile([NPROJ, CCW[ch]], BF16,
                                     name=f"ar_in{ch}_{rep}")
                         for ch in range(2)]
                ar_out = [drpool.tile([NPROJ, CCW[ch]], BF16,
                                      addr_space="Shared",
                                      name=f"ar_out{ch}_{rep}")
                          for ch in range(2)]
                res_silu = [ppool.tile([128, L], BF16,
                                       name=f"res_silu{i}_{rep}")
                            for i in range(NB)]
                rinv_bc = ppool.tile([128, L], F32, name=f"rinv_bc{rep}")

                # xs_pad lives only through A/B: right-side pool
                padpool_ctx = ExitStack()
                padpool = padpool_ctx.enter_context(
                    tc.tile_pool(name=f"pad{rep}", bufs=1, side="right"))
                xs_pad = [padpool.tile([128, L + KCONV - 1], BF16,
                                       name=f"xs_pad{i}_{rep}")
                          for i in range(NB)]
                for cb in range(NB):
                    nc.gpsimd.memset(xs_pad[cb][:, 0:KCONV - 1], 0.0)

                apool = rctx.enter_context(
                    tc.tile_pool(name=f"acts{rep}", bufs=1))
                xs_silu = [apool.tile([128, L], BF16, name=f"xs_silu{i}_{rep}")
                           for i in range(NB)]
                proj_sb = apool.tile([NPROJ, L], BF16, name=f"proj{rep}")

                # ================= Phase A =================
                # Load bf16 xT; sumsq via square + ones-matmul (lt-outer to
                # hold only 2 ss psum banks); rinv via Act Rsqrt table; main
                # matmul on UNSCALED xT, rinv applied on PSUM evacuation.
                with ExitStack() as actx:
                    xtpool = actx.enter_context(tc.tile_pool(name="xt", bufs=KB))
                    sqpool = actx.enter_context(tc.tile_pool(name="sq", bufs=3))
                    sspool = actx.enter_context(
                        tc.tile_pool(name="ps_ss", bufs=2, space="PSUM"))
                    mmpool = actx.enter_context(
                        tc.tile_pool(name="ps_mm", bufs=2, space="PSUM"))
                    rowpool = actx.enter_context(tc.tile_pool(name="rows", bufs=1))
                    evpool = actx.enter_context(tc.tile_pool(name="ev", bufs=3))

                    xt_ts = []
                    for kb in range(KB):
                        xt = xtpool.tile([128, L], BF16, tag="xt")
                        xt_ts.append(xt)
                    # lt0 columns first so the rinv chain and lt0's matmul
                    # (feeding the early cc0) start as soon as possible
                    for kb in range(KB):
                        nc.sync.dma_start(xt_ts[kb][:, 0:LTS],
                                          xT_d[bass.ts(kb, 128), 0:LTS])
                    for kb in range(KB):
                        nc.sync.dma_start(xt_ts[kb][:, LTS:L],
                                          xT_d[bass.ts(kb, 128), LTS:L])

                    rinv_row = rowpool.tile([1, L], F32)
                    rinv_rb = rowpool.tile([1, L], BF16)
                    for lt in range(NLT):
                        ls = bass.ts(lt, LTS)
                        ss_ps = sspool.tile([1, LTS], F32, tag="ss")
                        for kb in range(KB):
                            # x^2 on DVE in bf16 (2x mode) to offload Act
                            sq = sqpool.tile([128, LTS], BF16, tag="sq")
                            nc.vector.tensor_mul(
                                sq[:], xt_ts[kb][:, ls], xt_ts[kb][:, ls])
                            nc.tensor.matmul(
                                ss_ps[:], ones_t[:], sq[:],
                                start=(kb == 0), stop=(kb == KB - 1))
                        # rinv = 1/sqrt(ss/D + EPS): Act Sqrt + DVE recip
                        nc.scalar.activation(
                            rinv_row[:, ls], ss_ps[:],
                            AF.Sqrt, bias=eps_t[:], scale=1.0 / D)
                        with nc.allow_low_precision(
                                reason="rinv bf16 broadcast; 2e-2 tolerance"):
                            nc.vector.reciprocal(rinv_rb[:, ls],
                                                 rinv_row[:, ls])
                        # broadcast to 128 partitions via PE ones-matmul
                        # (avoids the DRAM round-trip latency)
                        rb_ps = sspool.tile([128, LTS], F32, tag="rbc", bufs=2)
                        nc.tensor.matmul(
                            rb_ps[:], onesr_t[:], rinv_rb[:, ls],
                            start=True, stop=True)
                        nc.scalar.copy(rinv_bc[:, ls], rb_ps[:])

                    # Fused per-lt pipeline: w_in matmul -> conv -> x_proj ->
                    # chunked AllReduce, so the first collective fires while
                    # later lt chunks are still in the matmul.
                    for lt in range(NLT):
                        ls = bass.ts(lt, LTS)
                        for m in range(2 * NB):
                            mm_ps = mmpool.tile([128, LTS], F32, tag="mm")
                            for kb in range(KB):
                                nc.tensor.matmul(
                                    mm_ps[:],
                                    w_in_t[:, kb * 512 + m * 128:
                                           kb * 512 + (m + 1) * 128],
                                    xt_ts[kb][:, ls],
                                    start=(kb == 0), stop=(kb == KB - 1))
                            if m < NB:
                                nc.vector.tensor_mul(
                                    xs_pad[m][:, KCONV - 1 + lt * LTS:
                                              KCONV - 1 + (lt + 1) * LTS],
                                    mm_ps[:], rinv_bc[:, ls])
                            else:
                                ev = evpool.tile([128, LTS], BF16, tag="ev")
                                nc.vector.tensor_mul(
                                    ev[:], mm_ps[:], rinv_bc[:, ls])
                                nc.scalar.activation(
                                    res_silu[m - NB][:, ls], ev[:], AF.Silu)
                        # conv + silu for this lt
                        for cb in range(NB):
                            c_ps = mmpool.tile([128, LTS], F32, tag="cps",
                                               bufs=1)
                            for j in range(KCONV):
                                nc.tensor.matmul(
                                    c_ps[:],
                                    cwdiag_t[:, (cb * KCONV + j) * 128:
                                             (cb * KCONV + j + 1) * 128],
                                    xs_pad[cb][:, j + lt * LTS:
                                               j + lt * LTS + LTS],
                                    start=(j == 0), stop=(j == KCONV - 1))
                            nc.scalar.activation(
                                xs_silu[cb][:, ls], c_ps[:],
                                AF.Silu, bias=cbias_t[:, cb:cb + 1])
                        # x_proj + AllReduce for this lt chunk
                        pr_ps = mmpool.tile([NPROJ, LTS], F32, tag="prps",
                                            bufs=1)
                        for cb in range(NB):
                            nc.tensor.matmul(
                                pr_ps[:],
                                wxp_t[:, cb * NPROJ:(cb + 1) * NPROJ],
                                xs_silu[cb][:, ls],
                                start=(cb == 0), stop=(cb == NB - 1))
                        nc.scalar.copy(proj_sb[:, ls], pr_ps[:])
                        if lt == LTPC - 1 or lt == NLT - 1:
                            ch = 0 if lt == LTPC - 1 else 1
                            off = ch * LC
                            nc.sync.dma_start(
                                ar_in[ch][:], proj_sb[:, off:off + CCW[ch]])
                            if collective:
                                nc.gpsimd.collective_compute(
                                    "AllReduce", OP.add,
                                    replica_groups=[list(range(CORES))],
                                    ins=[ar_in[ch].opt()],
                                    outs=[ar_out[ch].opt()])
                            else:
                                nc.sync.dma_start(ar_out[ch][:], ar_in[ch][:])

                    padpool_ctx.close()

                if stop_after in ("A", "B"):
                    raise _StopBuild()

                # ========== Phase C: delta, scan, y, fin, out_proj ==========
                # Chunked over NCH chunks of LC; scan state chained via a
                # [128, NB*NST] state tile.  Scans split DVE/Pool by pool_mod.
                state_t = rctx.enter_context(
                    tc.tile_pool(name=f"st{rep}", bufs=1)).tile(
                        [128, NB * NST], F32, name=f"state{rep}")

                with ExitStack() as cctx:
                    pdpool = cctx.enter_context(tc.tile_pool(name="pd", bufs=2))
                    dpool = cctx.enter_context(tc.tile_pool(name="dl", bufs=2))
                    upool = cctx.enter_context(tc.tile_pool(name="ul", bufs=2))
                    bcpool = cctx.enter_context(tc.tile_pool(name="bc", bufs=4))
                    dapool = cctx.enter_context(tc.tile_pool(name="da", bufs=4))
                    dxpool = cctx.enter_context(tc.tile_pool(name="dx", bufs=4))
                    yspool = cctx.enter_context(tc.tile_pool(name="ys", bufs=4))
                    zpool = cctx.enter_context(tc.tile_pool(name="zz", bufs=4))
                    fpool = cctx.enter_context(tc.tile_pool(name="fi", bufs=3))
                    dps_pool = cctx.enter_context(
                        tc.tile_pool(name="ps_d", bufs=2, space="PSUM"))
                    ypspool = cctx.enter_context(
                        tc.tile_pool(name="ps_y", bufs=1, space="PSUM"))
                    opspool = cctx.enter_context(
                        tc.tile_pool(name="ps_o", bufs=2, space="PSUM"))

                    # scan chunks: (t-start, width, collective chunk, offset)
                    SCH = [(0, LC, 0, 0), (LC, LC, 1, 0)]
                    for sc, (t0, W, ach, aoff) in enumerate(SCH):
                        cs = slice(t0, t0 + W)
                        WQ = W // LTS
                        # delta & u per channel block for this chunk
                        pd_sb = pdpool.tile([DTR, W], BF16, tag="pd")
                        nc.sync.dma_start(
                            pd_sb[:], ar_out[ach][0:DTR, aoff:aoff + W])
                        delta = []
                        u_t = []
                        for cb in range(NB):
                            dl = dpool.tile([128, W], F32, tag=f"dl{cb}")
                            for q in range(WQ):
                                qs = bass.ts(q, LTS)
                                d_ps = dps_pool.tile([128, LTS], F32, tag="dps")
                                nc.tensor.matmul(
                                    d_ps[:], wdt_t[:, bass.ts(cb, 128)],
                                    pd_sb[:, qs], start=True, stop=True)
                                # softplus(z) = z + ln(1 + exp(-z)); Exp and
                                # Ln share act-table set 6 with phase-C Exp.
                                e_t = dpool.tile([128, LTS], F32, tag="e")
                                nc.scalar.activation(
                                    e_t[:], d_ps[:], AF.Exp, scale=-1.0,
                                    bias=bdtn_t[:, cb:cb + 1])
                                t_t = dpool.tile([128, LTS], F32, tag="t")
                                nc.scalar.activation(
                                    t_t[:], e_t[:], AF.Ln, bias=1.0)
                                nc.vector.scalar_tensor_tensor(
                                    dl[:, qs], d_ps[:], bdt_t[:, cb:cb + 1],
                                    t_t[:], op0=OP.add, op1=OP.add)
                            delta.append(dl)
                            ul = upool.tile([128, W], BF16, tag=f"ul{cb}")
                            nc.vector.tensor_mul(
                                ul[:], dl[:], xs_silu[cb][:, cs])
                            u_t.append(ul)

                        y_ps = [[ypspool.tile([128, LTS], F32, tag=f"yps{cb}_{q}",
                                              name=f"yps{cb}_{q}_{sc}_{rep}")
                                 for q in range(WQ)] for cb in range(NB)]
                        # pre-charge y with xs*D via diag matmul: only needs
                        # xs_silu, so it runs while the collective is in
                        # flight; also shortens fin to a single multiply
                        for cb in range(NB):
                            for q in range(WQ):
                                lt = (t0 // LTS) + q
                                nc.tensor.matmul(
                                    y_ps[cb][q][:],
                                    ddiag_t[:, bass.ts(cb, 128)],
                                    xs_silu[cb][:, bass.ts(lt, LTS)],
                                    start=True, stop=False)

                        for n in range(NST):
                            Bb = bcpool.tile([128, W], BF16, tag="Bb")
                            Cb = bcpool.tile([128, W], BF16, tag="Cb")
                            nc.sync.dma_start(
                                Bb[:], ar_out[ach][DTR + n:DTR + n + 1,
                                                   aoff:aoff + W]
                                .partition_broadcast(128))
                            nc.sync.dma_start(
                                Cb[:], ar_out[ach][DTR + NST + n:
                                                   DTR + NST + n + 1,
                                                   aoff:aoff + W]
                                .partition_broadcast(128))
                            for cb in range(NB):
                                idx = n * NB + cb
                                da = dapool.tile([128, W], F32, tag="da")
                                nc.scalar.activation(
                                    da[:], delta[cb][:], AF.Exp,
                                    scale=Aneg_t[:, cb * NST + n:cb * NST + n + 1])
                                dbx = dxpool.tile([128, W], BF16, tag="dbx")
                                nc.vector.tensor_mul(dbx[:], u_t[cb][:], Bb[:])
                                ys = yspool.tile([128, W], BF16, tag="ys")
                                nc.vector.tensor_tensor_scan(
                                    ys[:], da[:], dbx[:],
                                    0.0 if sc == 0 else state_t[:, idx:idx + 1],
                                    op0=OP.mult, op1=OP.add)
                                if sc < len(SCH) - 1:
                                    nc.vector.tensor_copy(
                                        state_t[:, idx:idx + 1],
                                        ys[:, W - 1:W])
                                z = zpool.tile([128, W], BF16, tag="z")
                                zeng = (nc.gpsimd if idx % pool_mod != pool_mod - 1
                                        else nc.vector)
                                zeng.tensor_tensor(z[:], ys[:], Cb[:], op=OP.mult)
                                for q in range(WQ):
                                    nc.tensor.matmul(
                                        y_ps[cb][q][:], ident_t[:],
                                        z[:, bass.ts(q, LTS)],
                                        start=False, stop=(n == NST - 1))

                        # fin + out_proj for this chunk
                        fin = []
                        for cb in range(NB):
                            fl = fpool.tile([128, W], BF16, tag=f"fin{cb}")
                            for q in range(WQ):
                                lt = (t0 // LTS) + q
                                nc.vector.tensor_mul(
                                    fl[:, bass.ts(q, LTS)], y_ps[cb][q][:],
                                    res_silu[cb][:, bass.ts(lt, LTS)])
                            fin.append(fl)

                        if stop_after == "C" and sc == len(SCH) - 1:
                            raise _StopBuild()

                        for m in range(D // 128):
                            for q in range(WQ):
                                lt = (t0 // LTS) + q
                                o_ps = opspool.tile([128, LTS], F32, tag="ops")
                                for cb in range(NB):
                                    nc.tensor.matmul(
                                        o_ps[:],
                                        wout_t[:, cb * D + m * 128:
                                               cb * D + (m + 1) * 128],
                                        fin[cb][:, bass.ts(q, LTS)],
                                        start=(cb == 0), stop=(cb == NB - 1))
                                po = fpool.tile([128, LTS], BF16, tag="po")
                                nc.scalar.copy(po[:], o_ps[:])
                                nc.sync.dma_start(
                                    pout_d[bass.ts(m, 128), bass.ts(lt, LTS)],
                                    po[:])
        except _StopBuild:
            pass

    nc.compile()
    return nc


def _bf16(a):
    return np.asarray(a, dtype=mybir.dt.np(BF16))


def host_prep(inputs, L=2048):
    """Slice/replicate the full inputs into 8 per-core input maps."""
    x = np.asarray(inputs["x"], np.float32)
    norm_scale = np.asarray(inputs["norm_scale"], np.float32)
    w_in = np.asarray(inputs["w_in"], np.float32)
    conv_w = np.asarray(inputs["conv_w"], np.float32)
    conv_b = np.asarray(inputs["conv_b"], np.float32)
    A_log = np.asarray(inputs["A_log"], np.float32)
    D_in = np.asarray(inputs["D"], np.float32)
    w_xproj = np.asarray(inputs["w_xproj"], np.float32)
    w_dt = np.asarray(inputs["w_dt"], np.float32)
    b_dt = np.asarray(inputs["b_dt"], np.float32)
    w_out = np.asarray(inputs["w_out"], np.float32)

    x2 = x[0, :L, :]                              # (L, D)
    xT = np.ascontiguousarray(x2.T)               # (D, L)
    w_in_s = w_in * norm_scale[:, None]
    ident = np.eye(128, dtype=np.float32)
    KB = D // 128

    def pack_nb(v):                                # (CL,) -> [128, NB]
        return np.ascontiguousarray(v.reshape(NB, 128).T)

    in_maps = []
    for k in range(CORES):
        sl = slice(k * CL, (k + 1) * CL)
        wi = np.concatenate(
            [w_in_s[:, k * CL:(k + 1) * CL],
             w_in_s[:, DI + k * CL:DI + (k + 1) * CL]], axis=1)  # (D, 512)
        w_in_pack = np.ascontiguousarray(
            wi.reshape(KB, 128, 512).transpose(1, 0, 2).reshape(128, KB * 512))
        cw = conv_w[:, 0, sl]                     # (4, CL)
        A_pack = np.ascontiguousarray(
            A_log[sl].reshape(NB, 128, NST).transpose(1, 0, 2)
            .reshape(128, NB * NST))
        wxp_pack = np.ascontiguousarray(
            w_xproj[sl].reshape(NB, 128, DTR + 2 * NST)
            .transpose(1, 0, 2).reshape(128, NB * (DTR + 2 * NST)))
        wout_pack = np.ascontiguousarray(
            w_out[sl].reshape(NB, 128, D).transpose(1, 0, 2)
            .reshape(128, NB * D))
        in_maps.append({
            "xT": _bf16(xT),
            "w_in_pack": _bf16(w_in_pack),
            "cbias_pack": pack_nb(conv_b[sl]),
            "A_pack": A_pack,
            "D_pack": pack_nb(D_in[sl]),
            "wxp_pack": _bf16(wxp_pack),
            "wdt_loc": _bf16(np.ascontiguousarray(w_dt[:, sl])),
            "bdt_pack": pack_nb(b_dt[sl]),
            "bdtn_pack": pack_nb(-b_dt[sl]),
            "wout_pack": _bf16(wout_pack),
            "ident": _bf16(ident),
            "ones_in": _bf16(np.ones((128, 1), np.float32)),
            "cwdiag_pack": _bf16(np.concatenate(
                [np.diag(cw[j, cb * 128:(cb + 1) * 128]).astype(np.float32)
                 for cb in range(NB) for j in range(KCONV)], axis=1)),
            "ddiag_pack": _bf16(np.concatenate(
                [np.diag(D_in[sl][cb * 128:(cb + 1) * 128]).astype(np.float32)
                 for cb in range(NB)], axis=1)),
        })
    return in_maps


def combine(inputs, results, L=2048):
    """Host unshard: sum the 8 partial outputs, add residual."""
    x = np.asarray(inputs["x"], np.float32)
    acc = np.zeros((D, L), np.float32)
    for r in results:
        acc += np.asarray(r["part_out"], np.float32)
    out = x[0, :L, :] + acc.T
    return out[None].astype(np.float32)


_CACHE = {}


def kernel(**inputs):
    if "nc" not in _CACHE:
        _CACHE["nc"] = build()
    nc = _CACHE["nc"]
    in_maps = host_prep(inputs)
    res = run_bass_kernel_spmd(nc, in_maps, list(range(CORES)))
    return combine(inputs, res.results)


if __name__ == "__main__":
    import reference

    inputs = reference.setup_inputs()
    inputs = {k: np.asarray(v) for k, v in inputs.items()}
    expected = np.asarray(reference.reference(**inputs))
    actual = kernel(**inputs)
    err = np.abs(actual - expected).max() / np.abs(expected).max()
    print("Relative error:", err)


# revision 77
# speedup vs baseline: 1.0261x; 1.0261x over previous
"""Trainium2 Bass kernel for the Mamba-style block nn_Block_29721173688983.

Strategy: tensor-parallel over d_inner (2048 channels / 8 cores = 256 each).
Per core, one fused per-512-column pipeline: RMSNorm (redundant; sumsq via
bf16 DVE squares + ones-matmul, sqrt on Act, reciprocal on DVE, partition
broadcast via PE ones-matmul), bf16 w_in column-slice matmul, depthwise conv
as diagonal bf16 matmuls + silu, partial x_proj contraction -> two chunked
bf16 AllReduces (cc0 after half the sequence, pipelined with the rest).
Phase C per 1024-chunk: delta via softplus = z + ln(1+exp(-z)) (Exp/Ln share
one Act table set with the scan's exp), selective scan via DVE
tensor_tensor_scan (f32 da, bf16 dbx/ys, state chained across chunks),
z = ys*C split DVE/Pool, y = sum_n via bf16 identity-matmul PSUM
accumulation consumed per-chunk, fin + out_proj bf16 matmul streamed per
chunk (xs*D pre-charged into the y PSUM accumulation via a diag-D
matmul while the collective is in flight).  Host sums the 8 bf16 partial outputs in f32 and adds the residual.

kernel(**inputs) takes the FULL unsharded inputs from setup_inputs() and
returns the FULL (1, 2048, 1024) output.
"""

import sys

sys.path.insert(0, "/opt/trn_rl_repo")

from contextlib import ExitStack

import numpy as np

import concourse.bacc as bacc
import concourse.bass as bass
import concourse.tile as tile
from concourse import mybir
from concourse.bass_utils import run_bass_kernel_spmd

F32 = mybir.dt.float32
F32R = mybir.dt.float32r
BF16 = mybir.dt.bfloat16
AF = mybir.ActivationFunctionType
OP = mybir.AluOpType

CORES = 8
D = 1024
DI = 2048
CL = DI // CORES          # 256 channels per core
NB = CL // 128            # 2 channel blocks
NST = 16                  # d_state
DTR = 64                  # dt_rank
KCONV = 4
EPS = 1e-5
NPROJ = DTR + 2 * NST     # 96


class _StopBuild(Exception):
    pass


def build(L=2048, collective=True, stop_after="D", reps=1, pool_mod=6):
    """Emit the SPMD single-core program (same program on all 8 cores).

    pool_mod: z-mult (n,cb) goes to Pool unless idx % pool_mod == pool_mod-1.
    """
    LTS = 512                  # psum free-dim tile
    NLT = L // LTS             # 4
    KB = D // 128              # 8 k-blocks for the w_in matmul
    LC = min(1024, L)          # collective + scan chunk
    NCH = L // LC              # 2
    LPC = LC // LTS            # psum tiles per chunk (2)
    LTPC = LC // LTS           # lt chunks per collective chunk (2)

    nc = bacc.Bacc("TRN2", target_bir_lowering=False, debug=False,
                   num_devices=CORES if collective else 1)

    def din(name, shape, dt=F32):
        return nc.dram_tensor(name, shape, dt, kind="ExternalInput").ap()

    xT_d = din("xT", [D, L], BF16)
    w_in_d = din("w_in_pack", [128, KB * 512], BF16)        # [p, (kb, m*128)]
    cbias_d = din("cbias_pack", [128, NB])
    A_d = din("A_pack", [128, NB * NST])
    D_d = din("D_pack", [128, NB])
    wxp_d = din("wxp_pack", [128, NB * NPROJ], BF16)
    wdt_d = din("wdt_loc", [DTR, CL], BF16)
    bdt_d = din("bdt_pack", [128, NB])
    bdtn_d = din("bdtn_pack", [128, NB])
    wout_d = din("wout_pack", [128, NB * D], BF16)
    ident_d = din("ident", [128, 128], BF16)
    ones_d = din("ones_in", [128, 1], BF16)
    cwdiag_d = din("cwdiag_pack", [128, NB * KCONV * 128], BF16)
    ddiag_d = din("ddiag_pack", [128, NB * 128], BF16)

    pout_d = nc.dram_tensor("part_out", [D, L], BF16, kind="ExternalOutput").ap()

    with tile.TileContext(nc) as tc:
      with ExitStack() as ctx:
        try:
            # ---- persistent pools ----
            cpool = ctx.enter_context(tc.tile_pool(name="consts", bufs=1))
            drpool = ctx.enter_context(
                tc.tile_pool(name="dram", bufs=1, space="DRAM"))

            cbias_t = cpool.tile([128, NB], F32)
            A_t = cpool.tile([128, NB * NST], F32)
            Aneg_t = cpool.tile([128, NB * NST], F32)
            D_t = cpool.tile([128, NB], F32)
            wxp_t = cpool.tile([128, NB * NPROJ], BF16)
            wdt_t = cpool.tile([DTR, CL], BF16)
            bdt_t = cpool.tile([128, NB], F32)
            bdtn_t = cpool.tile([128, NB], F32)
            wout_t = cpool.tile([128, NB * D], BF16)
            ident_t = cpool.tile([128, 128], BF16)
            ones_t = cpool.tile([128, 1], BF16)
            onesr_t = cpool.tile([1, 128], BF16)
            eps_t = cpool.tile([1, 1], F32)
            cwdiag_t = cpool.tile([128, NB * KCONV * 128], BF16)
            ddiag_t = cpool.tile([128, NB * 128], BF16)
            w_in_t = cpool.tile([128, KB * 512], BF16)

            nc.sync.dma_start(cbias_t[:], cbias_d[:])
            nc.sync.dma_start(A_t[:], A_d[:])
            nc.sync.dma_start(D_t[:], D_d[:])
            nc.sync.dma_start(wxp_t[:], wxp_d[:])
            nc.sync.dma_start(wdt_t[:], wdt_d[:])
            nc.sync.dma_start(bdt_t[:], bdt_d[:])
            nc.sync.dma_start(bdtn_t[:], bdtn_d[:])
            nc.sync.dma_start(wout_t[:], wout_d[:])
            nc.sync.dma_start(ident_t[:], ident_d[:])
            nc.sync.dma_start(ones_t[:], ones_d[:])
            nc.sync.dma_start(cwdiag_t[:], cwdiag_d[:])
            nc.sync.dma_start(ddiag_t[:], ddiag_d[:])
            nc.sync.dma_start(w_in_t[:, 0:KB * 256], w_in_d[:, 0:KB * 256])
            nc.sync.dma_start(w_in_t[:, KB * 256:], w_in_d[:, KB * 256:])

            nc.vector.memset(eps_t[:], EPS)
            nc.vector.memset(onesr_t[:], 1.0)

            # A = -exp(A_log)
            nc.scalar.activation(Aneg_t[:], A_t[:], AF.Exp)
            nc.scalar.mul(Aneg_t[:], Aneg_t[:], -1.0)

            for rep in range(reps):
              with ExitStack() as rctx:
                # per-rep persistent activations (live through phase C)
                ppool = rctx.enter_context(
                    tc.tile_pool(name=f"persist{rep}", bufs=1))
                # symmetric collective chunks: cc0 = lt0..1, cc1 = lt2..3
                CCW = [LC, LC]
                ar_in = [drpool.tile([NPROJ, CCW[ch]], BF16,
                                     name=f"ar_in{ch}_{rep}")
                         for ch in range(2)]
                ar_out = [drpool.tile([NPROJ, CCW[ch]], BF16,
                                      addr_space="Shared",
                                      name=f"ar_out{ch}_{rep}")
                          for ch in range(2)]
                res_silu = [ppool.tile([128, L], BF16,
                                       name=f"res_silu{i}_{rep}")
                            for i in range(NB)]
                rinv_bc = ppool.tile([128, L], F32, name=f"rinv_bc{rep}")

                # xs_pad lives only through A/B: right-side pool
                padpool_ctx = ExitStack()
                padpool = padpool_ctx.enter_context(
                    tc.tile_pool(name=f"pad{rep}", bufs=1, side="right"))
                xs_pad = [padpool.tile([128, L + KCONV - 1], BF16,
                                       name=f"xs_pad{i}_{rep}")
                          for i in range(NB)]
                for cb in range(NB):
                    nc.gpsimd.memset(xs_pad[cb][:, 0:KCONV - 1], 0.0)

                apool = rctx.enter_context(
                    tc.tile_pool(name=f"acts{rep}", bufs=1))
                xs_silu = [apool.tile([128, L], BF16, name=f"xs_silu{i}_{rep}")
                           for i in range(NB)]
                proj_sb = apool.tile([NPROJ, L], BF16, name=f"proj{rep}")

                # ================= Phase A =================
                # Load bf16 xT; sumsq via square + ones-matmul (lt-outer to
                # hold only 2 ss psum banks); rinv via Act Rsqrt table; main
                # matmul on UNSCALED xT, rinv applied on PSUM evacuation.
                with ExitStack() as actx:
                    xtpool = actx.enter_context(tc.tile_pool(name="xt", bufs=KB))
                    sqpool = actx.enter_context(tc.tile_pool(name="sq", bufs=3))
                    sspool = actx.enter_context(
                        tc.tile_pool(name="ps_ss", bufs=2, space="PSUM"))
                    mmpool = actx.enter_context(
                        tc.tile_pool(name="ps_mm", bufs=2, space="PSUM"))
                    rowpool = actx.enter_context(tc.tile_pool(name="rows", bufs=1))
                    evpool = actx.enter_context(tc.tile_pool(name="ev", bufs=3))

                    xt_ts = []
                    for kb in range(KB):
                        xt = xtpool.tile([128, L], BF16, tag="xt")
                        xt_ts.append(xt)
                    # lt0 columns first so the rinv chain and lt0's matmul
                    # (feeding the early cc0) start as soon as possible
                    for kb in range(KB):
                        nc.sync.dma_start(xt_ts[kb][:, 0:LTS],
                                          xT_d[bass.ts(kb, 128), 0:LTS])
                    for kb in range(KB):
                        nc.sync.dma_start(xt_ts[kb][:, LTS:L],
                                          xT_d[bass.ts(kb, 128), LTS:L])

                    rinv_row = rowpool.tile([1, L], F32)
                    rinv_rb = rowpool.tile([1, L], BF16)
                    for lt in range(NLT):
                        ls = bass.ts(lt, LTS)
                        ss_ps = sspool.tile([1, LTS], F32, tag="ss")
                        for kb in range(KB):
                            # x^2 on DVE in bf16 (2x mode) to offload Act
                            sq = sqpool.tile([128, LTS], BF16, tag="sq")
                            nc.vector.tensor_mul(
                                sq[:], xt_ts[kb][:, ls], xt_ts[kb][:, ls])
                            nc.tensor.matmul(
                                ss_ps[:], ones_t[:], sq[:],
                                start=(kb == 0), stop=(kb == KB - 1))
                        # rinv = 1/sqrt(ss/D + EPS): Act Sqrt + DVE recip
                        nc.scalar.activation(
                            rinv_row[:, ls], ss_ps[:],
                            AF.Sqrt, bias=eps_t[:], scale=1.0 / D)
                        with nc.allow_low_precision(
                                reason="rinv bf16 broadcast; 2e-2 tolerance"):
                            nc.vector.reciprocal(rinv_rb[:, ls],
                                                 rinv_row[:, ls])
                        # broadcast to 128 partitions via PE ones-matmul
                        # (avoids the DRAM round-trip latency)
                        rb_ps = sspool.tile([128, LTS], F32, tag="rbc", bufs=2)
                        nc.tensor.matmul(
                            rb_ps[:], onesr_t[:], rinv_rb[:, ls],
                            start=True, stop=True)
                        nc.scalar.copy(rinv_bc[:, ls], rb_ps[:])

                    # Fused per-lt pipeline: w_in matmul -> conv -> x_proj ->
                    # chunked AllReduce, so the first collective fires while
                    # later lt chunks are still in the matmul.
                    for lt in range(NLT):
                        ls = bass.ts(lt, LTS)
                        for m in range(2 * NB):
                            mm_ps = mmpool.tile([128, LTS], F32, tag="mm")
                            for kb in range(KB):
                                nc.tensor.matmul(
                                    mm_ps[:],
                                    w_in_t[:, kb * 512 + m * 128:
                                           kb * 512 + (m + 1) * 128],
                                    xt_ts[kb][:, ls],
                                    start=(kb == 0), stop=(kb == KB - 1))
                            if m < NB:
                                nc.vector.tensor_mul(
                                    xs_pad[m][:, KCONV - 1 + lt * LTS:
                                              KCONV - 1 + (lt + 1) * LTS],
                                    mm_ps[:], rinv_bc[:, ls])
                            else:
                                ev = evpool.tile([128, LTS], BF16, tag="ev")
                                nc.vector.tensor_mul(
                                    ev[:], mm_ps[:], rinv_bc[:, ls])
                                nc.scalar.activation(
                                    res_silu[m - NB][:, ls], ev[:], AF.Silu)
                        # conv + silu for this lt
                        for cb in range(NB):
                            c_ps = mmpool.tile([128, LTS], F32, tag="cps",
                                               bufs=1)
                            for j in range(KCONV):
                                nc.tensor.matmul(
                                    c_ps[:],
                                    cwdiag_t[:, (cb * KCONV + j) * 128:
                                             (cb * KCONV + j + 1) * 128],
                                    xs_pad[cb][:, j + lt * LTS:
                                               j + lt * LTS + LTS],
                                    start=(j == 0), stop=(j == KCONV - 1))
                            nc.scalar.activation(
                                xs_silu[cb][:, ls], c_ps[:],
                                AF.Silu, bias=cbias_t[:, cb:cb + 1])
                        # x_proj + AllReduce for this lt chunk
                        pr_ps = mmpool.tile([NPROJ, LTS], F32, tag="prps",
                                            bufs=1)
                        for cb in range(NB):
                            nc.tensor.matmul(
                                pr_ps[:],
                                wxp_t[:, cb * NPROJ:(cb + 1) * NPROJ],
                                xs_silu[cb][:, ls],
                                start=(cb == 0), stop=(cb == NB - 1))
                        nc.scalar.copy(proj_sb[:, ls], pr_ps[:])
                        if lt == LTPC - 1 or lt == NLT - 1:
                            ch = 0 if lt == LTPC - 1 else 1
                            off = ch * LC
                            nc.sync.dma_start(
                                ar_in[ch][:], proj_sb[:, off:off + CCW[ch]])
                            if collective:
                                nc.gpsimd.collective_compute(
                                    "AllReduce", OP.add,
                                    replica_groups=[list(range(CORES))],
                                    ins=[ar_in[ch].opt()],
                                    outs=[ar_out[ch].opt()])
                            else:
                                nc.sync.dma_start(ar_out[ch][:], ar_in[ch][:])

                    padpool_ctx.close()

                if stop_after in ("A", "B"):
                    raise _StopBuild()

                # ========== Phase C: delta, scan, y, fin, out_proj ==========
                # Chunked over NCH chunks of LC; scan state chained via a
                # [128, NB*NST] state tile.  Scans split DVE/Pool by pool_mod.
                state_t = rctx.enter_context(
                    tc.tile_pool(name=f"st{rep}", bufs=1)).tile(
                        [128, NB * NST], F32, name=f"state{rep}")

                with ExitStack() as cctx:
                    pdpool = cctx.enter_context(tc.tile_pool(name="pd", bufs=2))
                    dpool = cctx.enter_context(tc.tile_pool(name="dl", bufs=2))
                    upool = cctx.enter_context(tc.tile_pool(name="ul", bufs=2))
                    bcpool = cctx.enter_context(tc.tile_pool(name="bc", bufs=4))
                    dapool = cctx.enter_context(tc.tile_pool(name="da", bufs=6))
                    dxpool = cctx.enter_context(tc.tile_pool(name="dx", bufs=6))
                    yspool = cctx.enter_context(tc.tile_pool(name="ys", bufs=6))
                    zpool = cctx.enter_context(tc.tile_pool(name="zz", bufs=6))
                    fpool = cctx.enter_context(tc.tile_pool(name="fi", bufs=3))
                    dps_pool = cctx.enter_context(
                        tc.tile_pool(name="ps_d", bufs=2, space="PSUM"))
                    ypspool = cctx.enter_context(
                        tc.tile_pool(name="ps_y", bufs=1, space="PSUM"))
                    opspool = cctx.enter_context(
                        tc.tile_pool(name="ps_o", bufs=2, space="PSUM"))

                    # scan chunks: (t-start, width, collective chunk, offset)
                    SCH = [(0, LC, 0, 0), (LC, LC, 1, 0)]
                    for sc, (t0, W, ach, aoff) in enumerate(SCH):
                        cs = slice(t0, t0 + W)
                        WQ = W // LTS
                        # delta & u per channel block for this chunk
                        pd_sb = pdpool.tile([DTR, W], BF16, tag="pd")
                        nc.sync.dma_start(
                            pd_sb[:], ar_out[ach][0:DTR, aoff:aoff + W])
                        delta = []
                        u_t = []
                        for cb in range(NB):
                            dl = dpool.tile([128, W], F32, tag=f"dl{cb}")
                            for q in range(WQ):
                                qs = bass.ts(q, LTS)
                                d_ps = dps_pool.tile([128, LTS], F32, tag="dps")
                                nc.tensor.matmul(
                                    d_ps[:], wdt_t[:, bass.ts(cb, 128)],
                                    pd_sb[:, qs], start=True, stop=True)
                                # softplus(z) = z + ln(1 + exp(-z)); Exp and
                                # Ln share act-table set 6 with phase-C Exp.
                                e_t = dpool.tile([128, LTS], F32, tag="e")
                                nc.scalar.activation(
                                    e_t[:], d_ps[:], AF.Exp, scale=-1.0,
                                    bias=bdtn_t[:, cb:cb + 1])
                                t_t = dpool.tile([128, LTS], F32, tag="t")
                                nc.scalar.activation(
                                    t_t[:], e_t[:], AF.Ln, bias=1.0)
                                nc.vector.scalar_tensor_tensor(
                                    dl[:, qs], d_ps[:], bdt_t[:, cb:cb + 1],
                                    t_t[:], op0=OP.add, op1=OP.add)
                            delta.append(dl)
                            ul = upool.tile([128, W], BF16, tag=f"ul{cb}")
                            nc.vector.tensor_mul(
                                ul[:], dl[:], xs_silu[cb][:, cs])
                            u_t.append(ul)

                        y_ps = [[ypspool.tile([128, LTS], F32, tag=f"yps{cb}_{q}",
                                              name=f"yps{cb}_{q}_{sc}_{rep}")
                                 for q in range(WQ)] for cb in range(NB)]
                        # pre-charge y with xs*D via diag matmul: only needs
                        # xs_silu, so it runs while the collective is in
                        # flight; also shortens fin to a single multiply
                        for cb in range(NB):
                            for q in range(WQ):
                                lt = (t0 // LTS) + q
                                nc.tensor.matmul(
                                    y_ps[cb][q][:],
                                    ddiag_t[:, bass.ts(cb, 128)],
                                    xs_silu[cb][:, bass.ts(lt, LTS)],
                                    start=True, stop=False)

                        for n in range(NST):
                            Bb = bcpool.tile([128, W], BF16, tag="Bb")
                            Cb = bcpool.tile([128, W], BF16, tag="Cb")
                            nc.sync.dma_start(
                                Bb[:], ar_out[ach][DTR + n:DTR + n + 1,
                                                   aoff:aoff + W]
                                .partition_broadcast(128))
                            nc.sync.dma_start(
                                Cb[:], ar_out[ach][DTR + NST + n:
                                                   DTR + NST + n + 1,
                                                   aoff:aoff + W]
                                .partition_broadcast(128))
                            for cb in range(NB):
                                idx = n * NB + cb
                                da = dapool.tile([128, W], F32, tag="da")
                                nc.scalar.activation(
                                    da[:], delta[cb][:], AF.Exp,
                                    scale=Aneg_t[:, cb * NST + n:cb * NST + n + 1])
                                dbx = dxpool.tile([128, W], BF16, tag="dbx")
                                nc.vector.tensor_mul(dbx[:], u_t[cb][:], Bb[:])
                                ys = yspool.tile([128, W], BF16, tag="ys")
                                nc.vector.tensor_tensor_scan(
                                    ys[:], da[:], dbx[:],
                                    0.0 if sc == 0 else state_t[:, idx:idx + 1],
                                    op0=OP.mult, op1=OP.add)
                                if sc < len(SCH) - 1:
                                    nc.vector.tensor_copy(
                                        state_t[:, idx:idx + 1],
                                        ys[:, W - 1:W])
                                z = zpool.tile([128, W], BF16, tag="z")
                                zeng = (nc.gpsimd if idx % pool_mod != pool_mod - 1
                                        else nc.vector)
                                zeng.tensor_tensor(z[:], ys[:], Cb[:], op=OP.mult)
                                for q in range(WQ):
                                    nc.tensor.matmul(
                                        y_ps[cb][q][:], ident_t[:],
                                        z[:, bass.ts(q, LTS)],
                                        start=False, stop=(n == NST - 1))

                        # fin + out_proj for this chunk
                        fin = []
                        for cb in range(NB):
                            fl = fpool.tile([128, W], BF16, tag=f"fin{cb}")
                            for q in range(WQ):
                                lt = (t0 // LTS) + q
                                nc.vector.tensor_mul(
                                    fl[:, bass.ts(q, LTS)], y_ps[cb][q][:],
                                    res_silu[cb][:, bass.ts(lt, LTS)])
                            fin.append(fl)

                        if stop_after == "C" and sc == len(SCH) - 1:
                            raise _StopBuild()

                        for m in range(D // 128):
                            for q in range(WQ):
                                lt = (t0 // LTS) + q
                                o_ps = opspool.tile([128, LTS], F32, tag="ops")
                                for cb in range(NB):
                                    nc.tensor.matmul(
                                        o_ps[:],
                                        wout_t[:, cb * D + m * 128:
                                               cb * D + (m + 1) * 128],
                                        fin[cb][:, bass.ts(q, LTS)],
                                        start=(cb == 0), stop=(cb == NB - 1))
                                po = fpool.tile([128, LTS], BF16, tag="po")
                                nc.scalar.copy(po[:], o_ps[:])
                                nc.sync.dma_start(
                                    pout_d[bass.ts(m, 128), bass.ts(lt, LTS)],
                                    po[:])
        except _StopBuild:
            pass

    nc.compile()
    return nc


def _bf16(a):
    return np.asarray(a, dtype=mybir.dt.np(BF16))


def host_prep(inputs, L=2048):
    """Slice/replicate the full inputs into 8 per-core input maps."""
    x = np.asarray(inputs["x"], np.float32)
    norm_scale = np.asarray(inputs["norm_scale"], np.float32)
    w_in = np.asarray(inputs["w_in"], np.float32)
    conv_w = np.asarray(inputs["conv_w"], np.float32)
    conv_b = np.asarray(inputs["conv_b"], np.float32)
    A_log = np.asarray(inputs["A_log"], np.float32)
    D_in = np.asarray(inputs["D"], np.float32)
    w_xproj = np.asarray(inputs["w_xproj"], np.float32)
    w_dt = np.asarray(inputs["w_dt"], np.float32)
    b_dt = np.asarray(inputs["b_dt"], np.float32)
    w_out = np.asarray(inputs["w_out"], np.float32)

    x2 = x[0, :L, :]                              # (L, D)
    xT = np.ascontiguousarray(x2.T)               # (D, L)
    w_in_s = w_in * norm_scale[:, None]
    ident = np.eye(128, dtype=np.float32)
    KB = D // 128

    def pack_nb(v):                                # (CL,) -> [128, NB]
        return np.ascontiguousarray(v.reshape(NB, 128).T)

    in_maps = []
    for k in range(CORES):
        sl = slice(k * CL, (k + 1) * CL)
        wi = np.concatenate(
            [w_in_s[:, k * CL:(k + 1) * CL],
             w_in_s[:, DI + k * CL:DI + (k + 1) * CL]], axis=1)  # (D, 512)
        w_in_pack = np.ascontiguousarray(
            wi.reshape(KB, 128, 512).transpose(1, 0, 2).reshape(128, KB * 512))
        cw = conv_w[:, 0, sl]                     # (4, CL)
        A_pack = np.ascontiguousarray(
            A_log[sl].reshape(NB, 128, NST).transpose(1, 0, 2)
            .reshape(128, NB * NST))
        wxp_pack = np.ascontiguousarray(
            w_xproj[sl].reshape(NB, 128, DTR + 2 * NST)
            .transpose(1, 0, 2).reshape(128, NB * (DTR + 2 * NST)))
        wout_pack = np.ascontiguousarray(
            w_out[sl].reshape(NB, 128, D).transpose(1, 0, 2)
            .reshape(128, NB * D))
        in_maps.append({
            "xT": _bf16(xT),
            "w_in_pack": _bf16(w_in_pack),
            "cbias_pack": pack_nb(conv_b[sl]),
            "A_pack": A_pack,
            "D_pack": pack_nb(D_in[sl]),
            "wxp_pack": _bf16(wxp_pack),
            "wdt_loc": _bf16(np.ascontiguousarray(w_dt[:, sl])),
            "bdt_pack": pack_nb(b_dt[sl]),
            "bdtn_pack": pack_nb(-b_dt[sl]),
            "wout_pack": _bf16(wout_pack),
            "ident": _bf16(ident),
            "ones_in": _bf16(np.ones((128, 1), np.float32)),
            "cwdiag_pack": _bf16(np.concatenate(
                [np.diag(cw[j, cb * 128:(cb + 1) * 128]).astype(np.float32)
                 for cb in range(NB) for j in range(KCONV)], axis=1)),
            "ddiag_pack": _bf16(np.concatenate(
                [np.diag(D_in[sl][cb * 128:(cb + 1) * 128]).astype(np.float32)
                 for cb in range(NB)], axis=1)),
        })
    return in_maps


def combine(inputs, results, L=2048):
    """Host unshard: sum the 8 partial outputs, add residual."""
    x = np.asarray(inputs["x"], np.float32)
    acc = np.zeros((D, L), np.float32)
    for r in results:
        acc += np.asarray(r["part_out"], np.float32)
    out = x[0, :L, :] + acc.T
    return out[None].astype(np.float32)


_CACHE = {}


def kernel(**inputs):
    if "nc" not in _CACHE:
        _CACHE["nc"] = build()
    nc = _CACHE["nc"]
    in_maps = host_prep(inputs)
    res = run_bass_kernel_spmd(nc, in_maps, list(range(CORES)))
    return combine(inputs, res.results)


if __name__ == "__main__":
    import reference

    inputs = reference.setup_inputs()
    inputs = {k: np.asarray(v) for k, v in inputs.items()}
    expected = np.asarray(reference.reference(**inputs))
    actual = kernel(**inputs)
    err = np.abs(actual - expected).max() / np.abs(expected).max()
    print("Relative error:", err)
